# revision 28
# baseline (speedup 1.0000x reference)
"""CosineCrossAttention Trainium2 kernel (fp8-hybrid).

Math (per (b,t)):
    q = query @ Wq                      (N, D), heads head-major: d = h*48+dh
    k = kv @ Wk   (1, D);  v = kv @ Wv  (1, D)
    attn[n,h] = (q_h . k_h) / (|q_h||k_h|)
    out[n, dh*8+h] = attn[n,h] * v[h,dh];  out = out @ Wp + bp

Restructured:
    k_scaled = k / |k_h|  per head (folded into tail via rnkT)
    Kmat[d,h]  = k[d] * (d//48 == h)                 (D, H)
    Wqk        = Wq @ Kmat                           (D, H)
    attn_raw   = query @ Wqk        = q.k            (N, H)   [bf16 streams]
    q~         = query @ (64*Wq)  in fp8 DoubleRow   (N, D)   [fp8 streams]
    ss[n,h]    = sum_{d in head h} (q~/64)[n,d]^2    via m48 fp8 matmul
    attn       = attn_raw * rnk[h] / sqrt(ss)
    v_perm     = kv @ Wv_perm   (Wv columns permuted so v_perm[d] = v[(d%8)*48+d//8])
    Wp_eff[h,:]= sum_d v_perm[d]*(d%8==h)*Wp[d,:]    (H, D)
    out        = [attn | 1] @ [Wp_eff ; bp]          (bias folded as 9th row)

Precision split: the q-projection runs in fp8e4m3 (+DoubleRow, 2 K-rows/cycle)
because its output only feeds the per-head norm, whose error averages down by
~sqrt(48).  attn_raw and the output projection stay bf16 (errors there hit the
output linearly).  Wq is pre-scaled x64 on the host to avoid fp8 subnormals;
the square-activation applies scale=1/64 before squaring to compensate.

Everything on-device runs in the transposed domain (D on partitions, N on
free).  Sharding: data-parallel over B across the 8 cores.  Output is stored
bf16 (halves store DMA) and upcast on the host.
"""

import sys

sys.path.insert(0, "/opt/trn_rl_repo")

from contextlib import ExitStack

import ml_dtypes
import numpy as np

import concourse.bass as bass
import concourse.tile as tile
from concourse import bacc, mybir
from concourse.masks import make_identity

F32 = mybir.dt.float32
BF16 = mybir.dt.bfloat16
FP8 = mybir.dt.float8e4

NP_BF16 = ml_dtypes.bfloat16
NP_FP8 = ml_dtypes.float8_e4m3

B, T, N, D, H, Dh = 8, 8, 2048, 384, 8, 48
P = 128
CH = D // P  # 3 chunks of the D dims
NG = 512  # n-group (one PSUM bank of f32)
WQ_SCALE = 64.0  # host pre-scales Wq by this before fp8 cast


def _contract384(nc, out, lhsT, rhs):
    """Contract 384 (=3x128) via one DoubleRow fp8 matmul (chunks 0,1)
    plus one plain fp8 matmul (chunk 2). lhsT/rhs are [128, 3, *].
    HW requires DR outputs to span all 128 partitions (col_grp=0xf) and the
    k-pair dim to have step%16==0 — callers must pad M to 128."""
    nc.tensor.matmul(
        out, lhsT[:, 0:2], rhs[:, 0:2],
        start=True, stop=False,
        perf_mode=mybir.MatmulPerfMode.DoubleRow,
    )
    nc.tensor.matmul(out, lhsT[:, 2], rhs[:, 2], start=False, stop=True)


def build_nc(t_dim=T, n_dim=N, ng=NG):
    ngrp = n_dim // ng
    nc = bacc.Bacc("TRN2", target_bir_lowering=False, debug=False)

    qT8 = nc.dram_tensor("qT8", [t_dim, D, n_dim], FP8, kind="ExternalInput").ap()
    qT16 = nc.dram_tensor("qT16", [t_dim, D, n_dim], BF16, kind="ExternalInput").ap()
    kvT = nc.dram_tensor("kvT", [D, t_dim], BF16, kind="ExternalInput").ap()
    wq8_d = nc.dram_tensor("Wq8", [D, D], FP8, kind="ExternalInput").ap()
    wqT_d = nc.dram_tensor("WqT", [D, D], BF16, kind="ExternalInput").ap()
    wk_d = nc.dram_tensor("Wk", [D, D], BF16, kind="ExternalInput").ap()
    wv_d = nc.dram_tensor("Wvp", [D, D], BF16, kind="ExternalInput").ap()
    wp_d = nc.dram_tensor("Wp", [D, D], BF16, kind="ExternalInput").ap()
    bpr_d = nc.dram_tensor("bpr", [t_dim, D], BF16, kind="ExternalInput").ap()
    m48_d = nc.dram_tensor("m48", [D, H], BF16, kind="ExternalInput").ap()
    # m48 padded to 128 output columns (cols 8:128 zero) so the DoubleRow
    # reduce matmul can legally span all 128 PSUM partitions
    m48_8_d = nc.dram_tensor("m48_8", [D, P], FP8, kind="ExternalInput").ap()
    mv_d = nc.dram_tensor("mv", [D, H], BF16, kind="ExternalInput").ap()
    outT = nc.dram_tensor("outT", [t_dim, D, n_dim], BF16, kind="ExternalOutput").ap()

    with tile.TileContext(nc) as tc, ExitStack() as ctx:
        consts = ctx.enter_context(tc.tile_pool(name="consts", bufs=1))
        qpool8 = ctx.enter_context(tc.tile_pool(name="qpool8", bufs=2))
        qpool16 = ctx.enter_context(tc.tile_pool(name="qpool16", bufs=2))
        work = ctx.enter_context(tc.tile_pool(name="work", bufs=2))
        small = ctx.enter_context(tc.tile_pool(name="small", bufs=4))
        qsqp = ctx.enter_context(tc.tile_pool(name="qsqp", bufs=2))
        osbp = ctx.enter_context(tc.tile_pool(name="osbp", bufs=2))
        # PSUM: pqA(2 banks) + pqB(1) + parA(1) + parB(1) + po(3) = 8 banks
        pqApool = ctx.enter_context(tc.tile_pool(name="pqApool", bufs=1, space="PSUM"))
        pqBpool = ctx.enter_context(tc.tile_pool(name="pqBpool", bufs=1, space="PSUM"))
        parApool = ctx.enter_context(tc.tile_pool(name="parApool", bufs=1, space="PSUM"))
        parBpool = ctx.enter_context(tc.tile_pool(name="parBpool", bufs=1, space="PSUM"))
        popool = ctx.enter_context(tc.tile_pool(name="popool", bufs=1, space="PSUM"))
        dram = ctx.enter_context(tc.tile_pool(name="dram", bufs=1, space="DRAM"))

        # ---------- hot-path constants first in the DMA queue ----------
        wq8_sb = consts.tile([P, CH, D], FP8, tag="wq8")
        nc.sync.dma_start(wq8_sb, wq8_d.rearrange("(c p) f -> p c f", p=P))

        # first query tiles (so qproj of group 0 can start during the prologue)
        qsplit = 2 if ngrp >= 2 else 1
        gph = ngrp // qsplit  # groups per query-slice tile
        nh = gph * ng

        def load_qt(pool, dramt, dt, t, hf, tag):
            qt = pool.tile([P, CH, nh], dt, tag=tag)
            for c in range(CH):
                nc.sync.dma_start(
                    qt[:, c, :],
                    dramt[t, c * P : (c + 1) * P, hf * nh : (hf + 1) * nh],
                )
            return qt

        qt8_0 = load_qt(qpool8, qT8, FP8, 0, 0, "qt8")

        # ---------- remaining constants (hot-path first) ----------
        def load_w(dramt, tag):
            sb = consts.tile([P, CH, D], BF16, tag=tag)
            nc.sync.dma_start(sb, dramt.rearrange("(c p) f -> p c f", p=P))
            return sb

        kvt_sb = consts.tile([P, CH, t_dim], BF16, tag="kvt")
        nc.sync.dma_start(kvt_sb, kvT.rearrange("(c p) t -> p c t", p=P))
        wk_sb = load_w(wk_d, "wk")
        wv_sb = load_w(wv_d, "wv")
        # first bf16 query tile: chunk 0 early (attn-c0 of group 0), the rest
        # after the prologue-critical constants
        qt16_0 = qpool16.tile([P, CH, nh], BF16, tag="qt16")
        nc.sync.dma_start(qt16_0[:, 0, :], qT16[0, 0:P, 0:nh])
        wqT_sb = load_w(wqT_d, "wqT")
        m48_sb = consts.tile([P, CH, H], BF16, tag="m48")
        nc.sync.dma_start(m48_sb, m48_d.rearrange("(c p) h -> p c h", p=P))
        mv_sb = consts.tile([P, CH, H], BF16, tag="mv")
        nc.sync.dma_start(mv_sb, mv_d.rearrange("(c p) h -> p c h", p=P))
        m48_8_sb = consts.tile([P, CH, P], FP8, tag="m48_8")
        nc.sync.dma_start(m48_8_sb, m48_8_d.rearrange("(c p) h -> p c h", p=P))
        for c in (1, 2):
            nc.sync.dma_start(
                qt16_0[:, c, :], qT16[0, c * P : (c + 1) * P, 0:nh]
            )
        wp_sb = load_w(wp_d, "wp")

        def emit_qproj(qt8, qsl):
            """q~ projection (fp8 DR) -> pqA (chunks 0,1) / pqB (chunk 2);
            split squares -> qsq (fp8).  Separate tiles per chunk-pair keep
            the WAR dependency of the next group's qproj on this group's
            square at sub-bank granularity."""
            pqA = pqApool.tile([P, 2, ng], F32, tag="pqA")
            pqB = pqBpool.tile([P, ng], F32, tag="pqB")
            for co in range(CH):
                dst = pqA[:, co, :] if co < 2 else pqB
                _contract384(nc, dst, wq8_sb[:, :, co * P : (co + 1) * P],
                             qt8[:, :, qsl])
            qsq = qsqp.tile([P, CH, ng], FP8, tag="qsq")
            nc.scalar.activation(
                qsq[:, 0:2], pqA, mybir.ActivationFunctionType.Square,
                bias=0.0, scale=1.0 / WQ_SCALE,
            )
            nc.scalar.activation(
                qsq[:, 2], pqB, mybir.ActivationFunctionType.Square,
                bias=0.0, scale=1.0 / WQ_SCALE,
            )
            return qsq

        # ---------- early q-projection for group 0 (overlaps prologue) ----------
        qsq0 = emit_qproj(qt8_0, slice(0, ng))

        # ---------- k/v projections for all t ----------
        ps_k = parApool.tile([t_dim, D], F32, tag="parA")
        ps_v = parBpool.tile([t_dim, D], F32, tag="parB")
        for c in range(CH):
            nc.tensor.matmul(
                ps_k, kvt_sb[:, c, :], wk_sb[:, c, :],
                start=(c == 0), stop=(c == CH - 1),
            )
        for c in range(CH):
            nc.tensor.matmul(
                ps_v, kvt_sb[:, c, :], wv_sb[:, c, :],
                start=(c == 0), stop=(c == CH - 1),
            )

        k_sb = work.tile([t_dim, D], F32, tag="k_sb")
        nc.scalar.copy(k_sb, ps_k)
        vsb = work.tile([t_dim, D], F32, tag="vsb")
        nc.scalar.copy(vsb, ps_v)

        # transpose k, v -> (D-part, t); cast to bf16 on copyback
        kT = consts.tile([P, CH, t_dim], BF16, tag="kT")
        vT = consts.tile([P, CH, t_dim], BF16, tag="vT")
        idt = consts.tile([t_dim, t_dim], F32, tag="idt")
        make_identity(nc, idt)
        for c in range(CH):
            pt = pqApool.tile([P, 2, ng], F32, tag="pqA")
            nc.tensor.transpose(pt[:, 0, 0:t_dim], k_sb[:, c * P : (c + 1) * P], idt)
            nc.vector.tensor_copy(kT[:, c, :], pt[:, 0, 0:t_dim])
            pt2 = popool.tile([P, CH, ng], F32, tag="po")
            nc.tensor.transpose(pt2[:, 0, 0:t_dim], vsb[:, c * P : (c + 1) * P], idt)
            nc.vector.tensor_copy(vT[:, c, :], pt2[:, 0, 0:t_dim])

        # per-head k norms: rnkT[h, t] = 1/|k_h|(t)
        ksqT = work.tile([P, CH, t_dim], BF16, tag="ksqT")
        nc.scalar.square(ksqT, kT)
        psk2 = parApool.tile([H, t_dim], F32, tag="parA")
        for c in range(CH):
            nc.tensor.matmul(
                psk2, m48_sb[:, c, :], ksqT[:, c, :],
                start=(c == 0), stop=(c == CH - 1),
            )
        rnkT = consts.tile([H, t_dim], F32, tag="rnkT")
        nc.scalar.sqrt(rnkT, psk2)
        nc.vector.reciprocal(rnkT, rnkT)

        # Kmat[d, t, h] = kT[d, t] * m48[d, h];  Vsel[d, t, h] = vT[d, t] * mv[d, h]
        kmat = consts.tile([P, CH, t_dim, H], BF16, tag="kmat")
        nc.vector.tensor_tensor(
            kmat,
            kT[:, :, :, None].to_broadcast((P, CH, t_dim, H)),
            m48_sb[:, :, None, :].to_broadcast((P, CH, t_dim, H)),
            op=mybir.AluOpType.mult,
        )
        vsel = consts.tile([P, CH, t_dim, H], BF16, tag="vsel")
        nc.vector.tensor_tensor(
            vsel,
            vT[:, :, :, None].to_broadcast((P, CH, t_dim, H)),
            mv_sb[:, :, None, :].to_broadcast((P, CH, t_dim, H)),
            op=mybir.AluOpType.mult,
        )

        # Wqk[d_in, t, h] = sum_dmid Wq[d_in, dmid] Kmat[dmid, t, h]
        wqk = consts.tile([P, CH, t_dim, H], BF16, tag="wqk")
        for ci in range(CH):
            pw = parBpool.tile([P, t_dim * H], F32, tag="parB")
            for cm in range(CH):
                nc.tensor.matmul(
                    pw,
                    wqT_sb[:, cm, ci * P : (ci + 1) * P],
                    kmat[:, cm, :, :],
                    start=(cm == 0), stop=(cm == CH - 1),
                )
            nc.scalar.copy(wqk[:, ci], pw.rearrange("p (t h) -> p t h", h=H))

        # Wp_eff[(t,h), d_out] = sum_d Vsel[d, t, h] * Wp[d, d_out]  (all t at once)
        pe_all = popool.tile([t_dim * H, CH * ng], F32, tag="po")
        for c in range(CH):
            nc.tensor.matmul(
                pe_all[:, 0:D], vsel[:, c].rearrange("p t h -> p (t h)"),
                wp_sb[:, c, :],
                start=(c == 0), stop=(c == CH - 1),
            )
        wpe_stage = work.tile([t_dim * H, D], BF16, tag="wpestage")
        nc.scalar.copy(wpe_stage, pe_all[:, 0:D])
        wpe_dram = dram.tile([t_dim * H, D], BF16)
        nc.sync.dma_start(wpe_dram, wpe_stage)
        # wpe' = [Wp_eff ; bp] : 9 rows per t (bias folded as row 8)
        wpe = consts.tile([H + 1, t_dim, D], BF16, tag="wpe")
        nc.sync.dma_start(wpe[0:H], wpe_dram.rearrange("(t h) d -> h t d", h=H))
        nc.sync.dma_start(wpe[H : H + 1], bpr_d.rearrange("t d -> (t d)"))

        # att' tiles: rows 0:8 written per group, row 8 = ones (for the bias row)
        natt = 3
        att_tiles = []
        for i in range(natt):
            at = consts.tile([H + 1, ng], BF16, tag=f"att{i}")
            nc.vector.memset(at, 1.0)
            att_tiles.append(at)

        # ---------- main loop ----------
        # 3-stage software pipeline; PE order per group:
        #   qproj-co0 | out-co0(g-2) | qproj-co1 | out-co1(g-2) | qproj-co2 |
        #   out-co2(g-2) | attn(g) | ss(g)
        # The out-mms (whose deps are 2 groups old) sit between the fp8 DR
        # pairs to hide their LDWEIGHTS; attn comes late so the parA WAR on
        # comb(g-1) has slack; squares run on Act during the interleave.
        def emit_norm_tail(parA, parB, att_t, t):
            nrm = small.tile([H, ng], F32, tag="nrm")
            nc.scalar.sqrt(nrm, parB[0:H, :])
            rcp = small.tile([H, ng], F32, tag="rcp")
            nc.vector.reciprocal_approx_fast(rcp, nrm)
            nc.vector.scalar_tensor_tensor(
                att_t[0:H, :], parA[0:H, :], rnkT[:, t : t + 1], rcp,
                op0=mybir.AluOpType.mult, op1=mybir.AluOpType.mult,
            )

        def emit_group(qt8, qt16, qsl, t, att_t, out_item, qsq=None,
                       tail_first=False):
            have_q = qsq is not None
            if not have_q:
                pqA = pqApool.tile([P, 2, ng], F32, tag="pqA")
                pqB = pqBpool.tile([P, ng], F32, tag="pqB")
                qsq = qsqp.tile([P, CH, ng], FP8, tag="qsq")
            if out_item is not None:
                att_p, t_p, sl_p = out_item
                po = popool.tile([P, CH, ng], F32, tag="po")
            for co in range(CH):
                if not have_q:
                    dst = pqA[:, co, :] if co < 2 else pqB
                    _contract384(nc, dst, wq8_sb[:, :, co * P : (co + 1) * P],
                                 qt8[:, :, qsl])
                    if co == 1:
                        nc.scalar.activation(
                            qsq[:, 0:2], pqA, mybir.ActivationFunctionType.Square,
                            bias=0.0, scale=1.0 / WQ_SCALE,
                        )
                    elif co == 2:
                        nc.scalar.activation(
                            qsq[:, 2], pqB, mybir.ActivationFunctionType.Square,
                            bias=0.0, scale=1.0 / WQ_SCALE,
                        )
                if out_item is not None:
                    nc.tensor.matmul(
                        po[:, co, :], wpe[:, t_p, co * P : (co + 1) * P], att_p,
                        start=True, stop=True,
                    )
            # attn_raw (bf16) -> parA rows 0:8
            parA = parApool.tile([H, ng], F32, tag="parA")
            for c in range(CH):
                nc.tensor.matmul(
                    parA, wqk[:, c, t, :], qt16[:, c, qsl],
                    start=(c == 0), stop=(c == CH - 1),
                )
            def emit_osb():
                if out_item is None:
                    return
                osb = osbp.tile([P, CH, ng], BF16, tag="osb")
                # split the PSUM->SBUF copy: 1/8 on Act, 7/8 on DVE
                ns = ng // 8
                nc.scalar.copy(osb[:, :, 0:ns], po[:, :, 0:ns])
                nc.vector.tensor_copy(osb[:, :, ns:ng], po[:, :, ns:ng])
                nc.sync.dma_start(
                    outT[t_p].rearrange("(c p) n -> p c n", p=P)[:, :, sl_p], osb
                )

            if not tail_first:
                emit_osb()
            # ss (fp8 DR, M padded to 128) -> parB rows 0:8
            parB = parBpool.tile([P, ng], F32, tag="parB")
            _contract384(nc, parB, m48_8_sb, qsq)
            emit_norm_tail(parA, parB, att_t, t)
            if tail_first:
                emit_osb()

        def emit_out_drain(att_p, t_p, sl_p, use_pq):
            # the second drain group reuses the (now idle) pq banks so the
            # two drains don't serialize on the po bank
            if use_pq:
                poA = pqApool.tile([P, 2, ng], F32, tag="pqA")
                poB = pqBpool.tile([P, ng], F32, tag="pqB")
                chunks = [poA[:, 0, :], poA[:, 1, :], poB]
            else:
                po = popool.tile([P, CH, ng], F32, tag="po")
                chunks = [po[:, co, :] for co in range(CH)]
            for co in range(CH):
                nc.tensor.matmul(
                    chunks[co], wpe[:, t_p, co * P : (co + 1) * P], att_p,
                    start=True, stop=True,
                )
            osb = osbp.tile([P, CH, ng], BF16, tag="osb")
            # drain: chunks 0-1 on Act, chunk 2 on DVE, DMA each part asap
            outv = outT[t_p].rearrange("(c p) n -> p c n", p=P)[:, :, sl_p]
            nc.scalar.copy(osb[:, 0, :], chunks[0])
            nc.scalar.copy(osb[:, 1, :], chunks[1])
            nc.sync.dma_start(outv[:, 0:2, :], osb[:, 0:2, :])
            nc.vector.tensor_copy(osb[:, 2, :], chunks[2])
            nc.sync.dma_start(outv[:, 2, :], osb[:, 2, :])

        pending = []
        gidx = 0
        for t in range(t_dim):
            for hf in range(qsplit):
                if t == 0 and hf == 0:
                    qt8, qt16 = qt8_0, qt16_0
                else:
                    qt8 = load_qt(qpool8, qT8, FP8, t, hf, "qt8")
                    qt16 = load_qt(qpool16, qT16, BF16, t, hf, "qt16")
                for gl in range(gph):
                    g = hf * gph + gl
                    sl = slice(g * ng, (g + 1) * ng)
                    qsl = slice(gl * ng, (gl + 1) * ng)
                    att_t = att_tiles[gidx % natt]
                    out_item = pending.pop(0) if len(pending) == 2 else None
                    first = t == 0 and hf == 0 and gl == 0
                    last = gidx == t_dim * ngrp - 1
                    emit_group(qt8, qt16, qsl, t, att_t, out_item,
                               qsq=qsq0 if first else None, tail_first=last)
                    pending.append((att_t, t, sl))
                    gidx += 1
        for i, p_ in enumerate(pending):
            emit_out_drain(*p_, use_pq=(i == 1))

    nc.compile()
    return nc


_CACHE = {}


def _get_nc(t_dim=T, n_dim=N):
    key = (t_dim, n_dim)
    if key not in _CACHE:
        _CACHE[key] = build_nc(t_dim, n_dim)
    return _CACHE[key]


def _host_prep(query, kv, Wq, Wk, Wv, Wp, bp):
    query = np.asarray(query, dtype=np.float32)
    kv = np.asarray(kv, dtype=np.float32)
    Wq = np.asarray(Wq, dtype=np.float32)
    Wk = np.ascontiguousarray(np.asarray(Wk, dtype=np.float32).astype(NP_BF16))
    Wv = np.asarray(Wv, dtype=np.float32)
    Wp = np.ascontiguousarray(np.asarray(Wp, dtype=np.float32).astype(NP_BF16))
    bp = np.asarray(bp, dtype=np.float32)

    b_dim, t_dim, n_dim, d = query.shape
    dh = d // H
    Wq8 = np.ascontiguousarray((Wq * WQ_SCALE).astype(NP_FP8))
    WqT = np.ascontiguousarray(Wq.T.astype(NP_BF16))
    # Wv with columns permuted: v_perm[d] = v[(d%H)*dh + d//H]
    perm = (np.arange(d) % H) * dh + np.arange(d) // H
    Wvp = np.ascontiguousarray(Wv[:, perm].astype(NP_BF16))
    dd = np.arange(d)
    hh = np.arange(H)
    m48f = (dd[:, None] // dh == hh[None, :]).astype(np.float32)
    m48 = m48f.astype(NP_BF16)
    # padded to 128 cols for the DoubleRow reduce (must span all PE columns)
    m48_8 = np.zeros((d, 128), dtype=NP_FP8)
    m48_8[:, :H] = m48f.astype(NP_FP8)
    mv = (dd[:, None] % H == hh[None, :]).astype(NP_BF16)
    bpr = np.ascontiguousarray(np.tile(bp[None, :], (t_dim, 1)).astype(NP_BF16))

    in_maps = []
    for b in range(b_dim):
        qTb = query[b].transpose(0, 2, 1)
        in_maps.append(
            {
                "qT8": np.ascontiguousarray(qTb.astype(NP_FP8)),
                "qT16": np.ascontiguousarray(qTb.astype(NP_BF16)),
                "kvT": np.ascontiguousarray(kv[b, :, 0, :].T.astype(NP_BF16)),
                "Wq8": Wq8,
                "WqT": WqT,
                "Wk": Wk,
                "Wvp": Wvp,
                "Wp": Wp,
                "bpr": bpr,
                "m48": m48,
                "m48_8": m48_8,
                "mv": mv,
            }
        )
    return in_maps, (b_dim, t_dim, n_dim, d)


def _gather(results, shape):
    b_dim, t_dim, n_dim, d = shape
    out = np.empty((b_dim, t_dim, n_dim, d), dtype=np.float32)
    for b in range(b_dim):
        out[b] = results[b]["outT"].astype(np.float32).transpose(0, 2, 1)
    return out


def kernel(query, kv, Wq, Wk, Wv, Wp, bp):
    from concourse.bass_utils import run_bass_kernel_spmd

    in_maps, shape = _host_prep(query, kv, Wq, Wk, Wv, Wp, bp)
    nc = _get_nc(shape[1], shape[2])
    res = run_bass_kernel_spmd(nc, in_maps, core_ids=list(range(len(in_maps))))
    return _gather(res.results, shape)


def _install_ntff_hook():
    """The agent image's antenv lacks axon_hooks; synthesize it so
    run_bass_kernel_spmd(trace=True) can capture NTFF profiles."""
    import types

    if "antenv.axon_hooks" in sys.modules:
        return
    sys.path.insert(0, "/root/.axon_site")
    from trn_agent_boot.trn_boot import _ntff_profile_via_ctypes

    hook = _ntff_profile_via_ctypes("/opt/axon/libaxon_pjrt.so")
    mod = types.ModuleType("antenv.axon_hooks")
    mod.get_axon_ntff_profile_hook = lambda: hook
    mod.set_axon_ntff_profile_hook = lambda h: None
    sys.modules["antenv.axon_hooks"] = mod


def kernel_traced(query, kv, Wq, Wk, Wv, Wp, bp):
    """Like kernel() but captures an NTFF profile; returns (out, results)."""
    from concourse.bass_utils import run_bass_kernel_spmd

    _install_ntff_hook()
    in_maps, shape = _host_prep(query, kv, Wq, Wk, Wv, Wp, bp)
    nc = _get_nc(shape[1], shape[2])
    res = run_bass_kernel_spmd(
        nc, in_maps, core_ids=list(range(len(in_maps))), trace=True
    )
    return _gather(res.results, shape), res


# revision 34
# speedup vs baseline: 1.0418x; 1.0418x over previous
"""CosineCrossAttention Trainium2 kernel (fp8-hybrid).

Math (per (b,t)):
    q = query @ Wq                      (N, D), heads head-major: d = h*48+dh
    k = kv @ Wk   (1, D);  v = kv @ Wv  (1, D)
    attn[n,h] = (q_h . k_h) / (|q_h||k_h|)
    out[n, dh*8+h] = attn[n,h] * v[h,dh];  out = out @ Wp + bp

Restructured:
    k_scaled = k / |k_h|  per head (folded into tail via rnkT)
    Kmat[d,h]  = k[d] * (d//48 == h)                 (D, H)
    Wqk        = Wq @ Kmat                           (D, H)
    attn_raw   = query @ Wqk        = q.k            (N, H)   [bf16 streams]
    q~         = query @ (64*Wq)  in fp8 DoubleRow   (N, D)   [fp8 streams]
    ss[n,h]    = sum_{d in head h} (q~/64)[n,d]^2    via m48 fp8 matmul
    attn       = attn_raw * rnk[h] / sqrt(ss)
    v_perm     = kv @ Wv_perm   (Wv columns permuted so v_perm[d] = v[(d%8)*48+d//8])
    Wp_eff[h,:]= sum_d v_perm[d]*(d%8==h)*Wp[d,:]    (H, D)
    out        = [attn | 1] @ [Wp_eff ; bp]          (bias folded as 9th row)

Precision split: the q-projection runs in fp8e4m3 (+DoubleRow, 2 K-rows/cycle)
because its output only feeds the per-head norm, whose error averages down by
~sqrt(48).  attn_raw and the output projection stay bf16 (errors there hit the
output linearly).  Wq is pre-scaled x64 on the host to avoid fp8 subnormals;
the square-activation applies scale=1/64 before squaring to compensate.

Everything on-device runs in the transposed domain (D on partitions, N on
free).  Sharding: data-parallel over B across the 8 cores.  Output is stored
bf16 (halves store DMA) and upcast on the host.
"""

import sys

sys.path.insert(0, "/opt/trn_rl_repo")

from contextlib import ExitStack

import ml_dtypes
import numpy as np

import concourse.bass as bass
import concourse.tile as tile
from concourse import bacc, mybir
from concourse.masks import make_identity

F32 = mybir.dt.float32
BF16 = mybir.dt.bfloat16
FP8 = mybir.dt.float8e4

NP_BF16 = ml_dtypes.bfloat16
NP_FP8 = ml_dtypes.float8_e4m3

B, T, N, D, H, Dh = 8, 8, 2048, 384, 8, 48
P = 128
CH = D // P  # 3 chunks of the D dims
NG = 512  # n-group (one PSUM bank of f32)
WQ_SCALE = 64.0  # host pre-scales Wq by this before fp8 cast


def _contract384(nc, out, lhsT, rhs):
    """Contract 384 (=3x128) via one DoubleRow fp8 matmul (chunks 0,1)
    plus one plain fp8 matmul (chunk 2). lhsT/rhs are [128, 3, *].
    HW requires DR outputs to span all 128 partitions (col_grp=0xf) and the
    k-pair dim to have step%16==0 — callers must pad M to 128."""
    nc.tensor.matmul(
        out, lhsT[:, 0:2], rhs[:, 0:2],
        start=True, stop=False,
        perf_mode=mybir.MatmulPerfMode.DoubleRow,
    )
    nc.tensor.matmul(out, lhsT[:, 2], rhs[:, 2], start=False, stop=True)


def build_nc(t_dim=T, n_dim=N, ng=NG):
    ngrp = n_dim // ng
    nc = bacc.Bacc("TRN2", target_bir_lowering=False, debug=False)

    qT8 = nc.dram_tensor("qT8", [t_dim, D, n_dim], FP8, kind="ExternalInput").ap()
    qT16 = nc.dram_tensor("qT16", [t_dim, D, n_dim], BF16, kind="ExternalInput").ap()
    kvT = nc.dram_tensor("kvT", [D, t_dim], BF16, kind="ExternalInput").ap()
    wq8_d = nc.dram_tensor("Wq8", [D, D], FP8, kind="ExternalInput").ap()
    wqT_d = nc.dram_tensor("WqT", [D, D], BF16, kind="ExternalInput").ap()
    wk_d = nc.dram_tensor("Wk", [D, D], BF16, kind="ExternalInput").ap()
    wv_d = nc.dram_tensor("Wvp", [D, D], BF16, kind="ExternalInput").ap()
    wp_d = nc.dram_tensor("Wp", [D, D], BF16, kind="ExternalInput").ap()
    bpr_d = nc.dram_tensor("bpr", [t_dim, D], BF16, kind="ExternalInput").ap()
    m48_d = nc.dram_tensor("m48", [D, H], BF16, kind="ExternalInput").ap()
    # m48 padded to 128 output columns (cols 8:128 zero) so the DoubleRow
    # reduce matmul can legally span all 128 PSUM partitions
    m48_8_d = nc.dram_tensor("m48_8", [D, P], FP8, kind="ExternalInput").ap()
    mv_d = nc.dram_tensor("mv", [D, H], BF16, kind="ExternalInput").ap()
    outT = nc.dram_tensor("outT", [t_dim, D, n_dim], BF16, kind="ExternalOutput").ap()

    with tile.TileContext(nc) as tc, ExitStack() as ctx:
        consts = ctx.enter_context(tc.tile_pool(name="consts", bufs=1))
        qpool8 = ctx.enter_context(tc.tile_pool(name="qpool8", bufs=2))
        qpool16 = ctx.enter_context(tc.tile_pool(name="qpool16", bufs=2))
        work = ctx.enter_context(tc.tile_pool(name="work", bufs=2))
        small = ctx.enter_context(tc.tile_pool(name="small", bufs=4))
        qsqp = ctx.enter_context(tc.tile_pool(name="qsqp", bufs=2))
        osbp = ctx.enter_context(tc.tile_pool(name="osbp", bufs=2))
        # PSUM: pq0/pq1/pq2 (1 bank each) + parA(1) + parB(1) + po(3) = 8 banks
        pqpools = [
            ctx.enter_context(tc.tile_pool(name=f"pq{i}pool", bufs=1, space="PSUM"))
            for i in range(CH)
        ]
        parApool = ctx.enter_context(tc.tile_pool(name="parApool", bufs=1, space="PSUM"))
        parBpool = ctx.enter_context(tc.tile_pool(name="parBpool", bufs=1, space="PSUM"))
        popool = ctx.enter_context(tc.tile_pool(name="popool", bufs=1, space="PSUM"))
        dram = ctx.enter_context(tc.tile_pool(name="dram", bufs=1, space="DRAM"))

        # ---------- hot-path constants first in the DMA queue ----------
        wq8_sb = consts.tile([P, CH, D], FP8, tag="wq8")
        nc.sync.dma_start(wq8_sb, wq8_d.rearrange("(c p) f -> p c f", p=P))

        # first query tiles (so qproj of group 0 can start during the prologue)
        qsplit = 2 if ngrp >= 2 else 1
        gph = ngrp // qsplit  # groups per query-slice tile
        nh = gph * ng

        def load_qt(pool, dramt, dt, t, hf, tag):
            qt = pool.tile([P, CH, nh], dt, tag=tag)
            for c in range(CH):
                nc.sync.dma_start(
                    qt[:, c, :],
                    dramt[t, c * P : (c + 1) * P, hf * nh : (hf + 1) * nh],
                )
            return qt

        qt8_0 = load_qt(qpool8, qT8, FP8, 0, 0, "qt8")

        # ---------- remaining constants (hot-path first) ----------
        def load_w(dramt, tag):
            sb = consts.tile([P, CH, D], BF16, tag=tag)
            nc.sync.dma_start(sb, dramt.rearrange("(c p) f -> p c f", p=P))
            return sb

        kvt_sb = consts.tile([P, CH, t_dim], BF16, tag="kvt")
        nc.sync.dma_start(kvt_sb, kvT.rearrange("(c p) t -> p c t", p=P))
        wk_sb = load_w(wk_d, "wk")
        wv_sb = load_w(wv_d, "wv")
        # first bf16 query tile: chunk 0 early (attn-c0 of group 0), the rest
        # after the prologue-critical constants
        qt16_0 = qpool16.tile([P, CH, nh], BF16, tag="qt16")
        nc.sync.dma_start(qt16_0[:, 0, :], qT16[0, 0:P, 0:nh])
        wqT_sb = load_w(wqT_d, "wqT")
        m48_sb = consts.tile([P, CH, H], BF16, tag="m48")
        nc.sync.dma_start(m48_sb, m48_d.rearrange("(c p) h -> p c h", p=P))
        mv_sb = consts.tile([P, CH, H], BF16, tag="mv")
        nc.sync.dma_start(mv_sb, mv_d.rearrange("(c p) h -> p c h", p=P))
        m48_8_sb = consts.tile([P, CH, P], FP8, tag="m48_8")
        nc.sync.dma_start(m48_8_sb, m48_8_d.rearrange("(c p) h -> p c h", p=P))
        for c in (1, 2):
            nc.sync.dma_start(
                qt16_0[:, c, :], qT16[0, c * P : (c + 1) * P, 0:nh]
            )
        wp_sb = load_w(wp_d, "wp")

        def emit_qproj(qt8, qsl):
            """q~ projection (fp8 DR) -> pq0/pq1/pq2 (one bank each);
            per-chunk squares -> qsq (fp8).  Separate tiles per chunk keep
            the WAR of the next group's qproj-co on this group's square-co
            at bank granularity, so the PE queue never drains."""
            qsq = qsqp.tile([P, CH, ng], FP8, tag="qsq")
            for co in range(CH):
                pq = pqpools[co].tile([P, ng], F32, tag=f"pq{co}")
                _contract384(nc, pq, wq8_sb[:, :, co * P : (co + 1) * P],
                             qt8[:, :, qsl])
                nc.scalar.activation(
                    qsq[:, co], pq, mybir.ActivationFunctionType.Square,
                    bias=0.0, scale=1.0 / WQ_SCALE,
                )
            return qsq

        # ---------- early q-projection for group 0 (overlaps prologue) ----------
        qsq0 = emit_qproj(qt8_0, slice(0, ng))

        # ---------- k/v projections for all t ----------
        ps_k = parApool.tile([t_dim, D], F32, tag="parA")
        ps_v = parBpool.tile([t_dim, D], F32, tag="parB")
        for c in range(CH):
            nc.tensor.matmul(
                ps_k, kvt_sb[:, c, :], wk_sb[:, c, :],
                start=(c == 0), stop=(c == CH - 1),
            )
        for c in range(CH):
            nc.tensor.matmul(
                ps_v, kvt_sb[:, c, :], wv_sb[:, c, :],
                start=(c == 0), stop=(c == CH - 1),
            )

        k_sb = work.tile([t_dim, D], F32, tag="k_sb")
        nc.scalar.copy(k_sb, ps_k)
        vsb = work.tile([t_dim, D], F32, tag="vsb")
        nc.scalar.copy(vsb, ps_v)

        # transpose k, v -> (D-part, t); cast to bf16 on copyback
        kT = consts.tile([P, CH, t_dim], BF16, tag="kT")
        vT = consts.tile([P, CH, t_dim], BF16, tag="vT")
        idt = consts.tile([t_dim, t_dim], F32, tag="idt")
        make_identity(nc, idt)
        for c in range(CH):
            pt = pqpools[0].tile([P, ng], F32, tag="pq0")
            nc.tensor.transpose(pt[:, 0:t_dim], k_sb[:, c * P : (c + 1) * P], idt)
            nc.vector.tensor_copy(kT[:, c, :], pt[:, 0:t_dim])
            pt2 = popool.tile([P, CH, ng], F32, tag="po")
            nc.tensor.transpose(pt2[:, 0, 0:t_dim], vsb[:, c * P : (c + 1) * P], idt)
            nc.vector.tensor_copy(vT[:, c, :], pt2[:, 0, 0:t_dim])

        # per-head k norms: rnkT[h, t] = 1/|k_h|(t)
        ksqT = work.tile([P, CH, t_dim], BF16, tag="ksqT")
        nc.scalar.square(ksqT, kT)
        psk2 = parApool.tile([H, t_dim], F32, tag="parA")
        for c in range(CH):
            nc.tensor.matmul(
                psk2, m48_sb[:, c, :], ksqT[:, c, :],
                start=(c == 0), stop=(c == CH - 1),
            )
        rnkT = consts.tile([H, t_dim], F32, tag="rnkT")
        nc.scalar.sqrt(rnkT, psk2)
        nc.vector.reciprocal(rnkT, rnkT)

        # Kmat[d, t, h] = kT[d, t] * m48[d, h];  Vsel[d, t, h] = vT[d, t] * mv[d, h]
        kmat = consts.tile([P, CH, t_dim, H], BF16, tag="kmat")
        nc.vector.tensor_tensor(
            kmat,
            kT[:, :, :, None].to_broadcast((P, CH, t_dim, H)),
            m48_sb[:, :, None, :].to_broadcast((P, CH, t_dim, H)),
            op=mybir.AluOpType.mult,
        )
        vsel = consts.tile([P, CH, t_dim, H], BF16, tag="vsel")
        nc.vector.tensor_tensor(
            vsel,
            vT[:, :, :, None].to_broadcast((P, CH, t_dim, H)),
            mv_sb[:, :, None, :].to_broadcast((P, CH, t_dim, H)),
            op=mybir.AluOpType.mult,
        )

        # Wqk[d_in, t, h] = sum_dmid Wq[d_in, dmid] Kmat[dmid, t, h]
        wqk = consts.tile([P, CH, t_dim, H], BF16, tag="wqk")
        for ci in range(CH):
            pw = parBpool.tile([P, t_dim * H], F32, tag="parB")
            for cm in range(CH):
                nc.tensor.matmul(
                    pw,
                    wqT_sb[:, cm, ci * P : (ci + 1) * P],
                    kmat[:, cm, :, :],
                    start=(cm == 0), stop=(cm == CH - 1),
                )
            nc.scalar.copy(wqk[:, ci], pw.rearrange("p (t h) -> p t h", h=H))

        # Wp_eff[(t,h), d_out] = sum_d Vsel[d, t, h] * Wp[d, d_out]  (all t at once)
        pe_all = popool.tile([t_dim * H, CH * ng], F32, tag="po")
        for c in range(CH):
            nc.tensor.matmul(
                pe_all[:, 0:D], vsel[:, c].rearrange("p t h -> p (t h)"),
                wp_sb[:, c, :],
                start=(c == 0), stop=(c == CH - 1),
            )
        wpe_stage = work.tile([t_dim * H, D], BF16, tag="wpestage")
        nc.scalar.copy(wpe_stage, pe_all[:, 0:D])
        wpe_dram = dram.tile([t_dim * H, D], BF16)
        nc.sync.dma_start(wpe_dram, wpe_stage)
        # wpe' = [Wp_eff ; bp] : 9 rows per t (bias folded as row 8)
        wpe = consts.tile([H + 1, t_dim, D], BF16, tag="wpe")
        nc.sync.dma_start(wpe[0:H], wpe_dram.rearrange("(t h) d -> h t d", h=H))
        nc.sync.dma_start(wpe[H : H + 1], bpr_d.rearrange("t d -> (t d)"))

        # att' tiles: rows 0:8 written per group, row 8 = ones (for the bias row)
        natt = 3
        att_tiles = []
        for i in range(natt):
            at = consts.tile([H + 1, ng], BF16, tag=f"att{i}")
            nc.vector.memset(at, 1.0)
            att_tiles.append(at)

        # ---------- main loop ----------
        # 3-stage software pipeline; PE order per group:
        #   qproj-co0 | out-co0(g-2) | qproj-co1 | out-co1(g-2) | qproj-co2 |
        #   out-co2(g-2) | attn(g) | ss(g)
        # The out-mms (whose deps are 2 groups old) sit between the fp8 DR
        # pairs to hide their LDWEIGHTS; attn comes late so the parA WAR on
        # comb(g-1) has slack; squares run on Act during the interleave.
        def emit_norm_tail(parA, parB, att_t, t):
            nrm = small.tile([H, ng], F32, tag="nrm")
            nc.scalar.sqrt(nrm, parB[0:H, :])
            rcp = small.tile([H, ng], F32, tag="rcp")
            nc.vector.reciprocal_approx_fast(rcp, nrm)
            nc.vector.scalar_tensor_tensor(
                att_t[0:H, :], parA[0:H, :], rnkT[:, t : t + 1], rcp,
                op0=mybir.AluOpType.mult, op1=mybir.AluOpType.mult,
            )

        def emit_group(qt8, qt16, qsl, t, att_t, out_item, qsq=None,
                       tail_first=False):
            have_q = qsq is not None
            if not have_q:
                qsq = qsqp.tile([P, CH, ng], FP8, tag="qsq")
            if out_item is not None:
                att_p, t_p, sl_p = out_item
                po = popool.tile([P, CH, ng], F32, tag="po")
            for co in range(CH):
                if not have_q:
                    pq = pqpools[co].tile([P, ng], F32, tag=f"pq{co}")
                    _contract384(nc, pq, wq8_sb[:, :, co * P : (co + 1) * P],
                                 qt8[:, :, qsl])
                    nc.scalar.activation(
                        qsq[:, co], pq, mybir.ActivationFunctionType.Square,
                        bias=0.0, scale=1.0 / WQ_SCALE,
                    )
                if out_item is not None:
                    nc.tensor.matmul(
                        po[:, co, :], wpe[:, t_p, co * P : (co + 1) * P], att_p,
                        start=True, stop=True,
                    )
            # attn_raw (bf16) -> parA rows 0:8
            parA = parApool.tile([H, ng], F32, tag="parA")
            for c in range(CH):
                nc.tensor.matmul(
                    parA, wqk[:, c, t, :], qt16[:, c, qsl],
                    start=(c == 0), stop=(c == CH - 1),
                )
            def emit_osb():
                if out_item is None:
                    return
                osb = osbp.tile([P, CH, ng], BF16, tag="osb")
                # split the PSUM->SBUF copy: 1/8 on Act, 7/8 on DVE
                ns = ng // 8
                nc.scalar.copy(osb[:, :, 0:ns], po[:, :, 0:ns])
                nc.vector.tensor_copy(osb[:, :, ns:ng], po[:, :, ns:ng])
                nc.sync.dma_start(
                    outT[t_p].rearrange("(c p) n -> p c n", p=P)[:, :, sl_p], osb
                )

            if not tail_first:
                emit_osb()
            # ss (fp8 DR, M padded to 128) -> parB rows 0:8
            parB = parBpool.tile([P, ng], F32, tag="parB")
            _contract384(nc, parB, m48_8_sb, qsq)
            emit_norm_tail(parA, parB, att_t, t)
            if tail_first:
                emit_osb()

        def emit_out_drain(att_p, t_p, sl_p, use_pq):
            # the second drain group reuses the (now idle) pq banks so the
            # two drains don't serialize on the po bank
            if use_pq:
                chunks = []
                for co in range(CH):
                    podr = pqpools[co].tile([P, ng], F32, tag=f"pq{co}")
                    chunks.append(podr)
            else:
                po = popool.tile([P, CH, ng], F32, tag="po")
                chunks = [po[:, co, :] for co in range(CH)]
            for co in range(CH):
                nc.tensor.matmul(
                    chunks[co], wpe[:, t_p, co * P : (co + 1) * P], att_p,
                    start=True, stop=True,
                )
            osb = osbp.tile([P, CH, ng], BF16, tag="osb")
            # drain: chunks 0-1 on Act, chunk 2 on DVE, DMA each part asap
            outv = outT[t_p].rearrange("(c p) n -> p c n", p=P)[:, :, sl_p]
            nc.scalar.copy(osb[:, 0, :], chunks[0])
            nc.scalar.copy(osb[:, 1, :], chunks[1])
            nc.sync.dma_start(outv[:, 0:2, :], osb[:, 0:2, :])
            nc.vector.tensor_copy(osb[:, 2, :], chunks[2])
            nc.sync.dma_start(outv[:, 2, :], osb[:, 2, :])

        pending = []
        gidx = 0
        for t in range(t_dim):
            for hf in range(qsplit):
                if t == 0 and hf == 0:
                    qt8, qt16 = qt8_0, qt16_0
                else:
                    qt8 = load_qt(qpool8, qT8, FP8, t, hf, "qt8")
                    qt16 = load_qt(qpool16, qT16, BF16, t, hf, "qt16")
                for gl in range(gph):
                    g = hf * gph + gl
                    sl = slice(g * ng, (g + 1) * ng)
                    qsl = slice(gl * ng, (gl + 1) * ng)
                    att_t = att_tiles[gidx % natt]
                    out_item = pending.pop(0) if len(pending) == 2 else None
                    first = t == 0 and hf == 0 and gl == 0
                    last = gidx == t_dim * ngrp - 1
                    emit_group(qt8, qt16, qsl, t, att_t, out_item,
                               qsq=qsq0 if first else None, tail_first=last)
                    pending.append((att_t, t, sl))
                    gidx += 1
        for i, p_ in enumerate(pending):
            emit_out_drain(*p_, use_pq=(i == 1))

    nc.compile()
    return nc


_CACHE = {}


def _get_nc(t_dim=T, n_dim=N):
    key = (t_dim, n_dim)
    if key not in _CACHE:
        _CACHE[key] = build_nc(t_dim, n_dim)
    return _CACHE[key]


def _host_prep(query, kv, Wq, Wk, Wv, Wp, bp):
    query = np.asarray(query, dtype=np.float32)
    kv = np.asarray(kv, dtype=np.float32)
    Wq = np.asarray(Wq, dtype=np.float32)
    Wk = np.ascontiguousarray(np.asarray(Wk, dtype=np.float32).astype(NP_BF16))
    Wv = np.asarray(Wv, dtype=np.float32)
    Wp = np.ascontiguousarray(np.asarray(Wp, dtype=np.float32).astype(NP_BF16))
    bp = np.asarray(bp, dtype=np.float32)

    b_dim, t_dim, n_dim, d = query.shape
    dh = d // H
    Wq8 = np.ascontiguousarray((Wq * WQ_SCALE).astype(NP_FP8))
    WqT = np.ascontiguousarray(Wq.T.astype(NP_BF16))
    # Wv with columns permuted: v_perm[d] = v[(d%H)*dh + d//H]
    perm = (np.arange(d) % H) * dh + np.arange(d) // H
    Wvp = np.ascontiguousarray(Wv[:, perm].astype(NP_BF16))
    dd = np.arange(d)
    hh = np.arange(H)
    m48f = (dd[:, None] // dh == hh[None, :]).astype(np.float32)
    m48 = m48f.astype(NP_BF16)
    # padded to 128 cols for the DoubleRow reduce (must span all PE columns)
    m48_8 = np.zeros((d, 128), dtype=NP_FP8)
    m48_8[:, :H] = m48f.astype(NP_FP8)
    mv = (dd[:, None] % H == hh[None, :]).astype(NP_BF16)
    bpr = np.ascontiguousarray(np.tile(bp[None, :], (t_dim, 1)).astype(NP_BF16))

    in_maps = []
    for b in range(b_dim):
        qTb = query[b].transpose(0, 2, 1)
        in_maps.append(
            {
                "qT8": np.ascontiguousarray(qTb.astype(NP_FP8)),
                "qT16": np.ascontiguousarray(qTb.astype(NP_BF16)),
                "kvT": np.ascontiguousarray(kv[b, :, 0, :].T.astype(NP_BF16)),
                "Wq8": Wq8,
                "WqT": WqT,
                "Wk": Wk,
                "Wvp": Wvp,
                "Wp": Wp,
                "bpr": bpr,
                "m48": m48,
                "m48_8": m48_8,
                "mv": mv,
            }
        )
    return in_maps, (b_dim, t_dim, n_dim, d)


def _gather(results, shape):
    b_dim, t_dim, n_dim, d = shape
    out = np.empty((b_dim, t_dim, n_dim, d), dtype=np.float32)
    for b in range(b_dim):
        out[b] = results[b]["outT"].astype(np.float32).transpose(0, 2, 1)
    return out


def kernel(query, kv, Wq, Wk, Wv, Wp, bp):
    from concourse.bass_utils import run_bass_kernel_spmd

    in_maps, shape = _host_prep(query, kv, Wq, Wk, Wv, Wp, bp)
    nc = _get_nc(shape[1], shape[2])
    res = run_bass_kernel_spmd(nc, in_maps, core_ids=list(range(len(in_maps))))
    return _gather(res.results, shape)


def _install_ntff_hook():
    """The agent image's antenv lacks axon_hooks; synthesize it so
    run_bass_kernel_spmd(trace=True) can capture NTFF profiles."""
    import types

    if "antenv.axon_hooks" in sys.modules:
        return
    sys.path.insert(0, "/root/.axon_site")
    from trn_agent_boot.trn_boot import _ntff_profile_via_ctypes

    hook = _ntff_profile_via_ctypes("/opt/axon/libaxon_pjrt.so")
    mod = types.ModuleType("antenv.axon_hooks")
    mod.get_axon_ntff_profile_hook = lambda: hook
    mod.set_axon_ntff_profile_hook = lambda h: None
    sys.modules["antenv.axon_hooks"] = mod


def kernel_traced(query, kv, Wq, Wk, Wv, Wp, bp):
    """Like kernel() but captures an NTFF profile; returns (out, results)."""
    from concourse.bass_utils import run_bass_kernel_spmd

    _install_ntff_hook()
    in_maps, shape = _host_prep(query, kv, Wq, Wk, Wv, Wp, bp)
    nc = _get_nc(shape[1], shape[2])
    res = run_bass_kernel_spmd(
        nc, in_maps, core_ids=list(range(len(in_maps))), trace=True
    )
    return _gather(res.results, shape), res
